# revision 27
# baseline (speedup 1.0000x reference)
"""BitLinear (ternary-quantized linear) Trainium2 kernel, v3.

Computes: scale = clip(mean(|w|, axis=1), 1e-5);  w_q = clip(round(w/scale), -1, 1)
          out = x @ (w_q * scale).T
for x [4, 2048, 2048] f32, w [8192, 2048] f32, out [4, 2048, 8192] f32.

Strategy (8 NeuronCores in a 2x4 token-half x out-quarter grid):
  - Core (th, oq) computes out features [oq*2048, (oq+1)*2048) for tokens
    [th*4096, (th+1)*4096): x traffic halves vs pure output sharding; total
    per-core HBM traffic is 32(x) + 32(w both layouts) + 16(out bf16) = 80 MB.
  - Host feeds x transposed [d_in, tokens] and the w shard in BOTH layouts
    ([o, k] for the bit-exact per-row scale reduction on the DVE, [k, o] for
    quantization in matmul orientation) - layout choices only; all
    arithmetic happens on device.
  - Per-row scale uses the blocked-512 two-stage reduction (bit-exact match
    with the reference's jnp.mean lowering); w_q = (w > s/2) - (w < -s/2)
    equals clip(round(w/scale), -1, 1) exactly. Ternary w_q is exact in
    fp8e4 and bf16.
  - The contraction (2048 = 16 chunks of 128) is split: the first BL_FP8_KC
    (default 14) chunks run as fp8e4 DoubleRow matmuls (2 chunks/matmul, 2x
    PE rate), the rest as bf16 matmuls. x is cast f32->fp8e4/bf16 on the
    scalar engine per 512-token slab. Measured end-to-end error ~1.9e-2
    scale-relative (gate 2e-2).
  - Matmuls run with w_q stationary and x moving, so PSUM is [o, tokens]
    and the per-o scale is per-PARTITION: the epilogue is a scalar-engine
    activation copy (fast PSUM reads), keeping the vector engine free for
    quantization. Output is stored bf16 in [o, t] layout (contiguous rows,
    one 2 MB store per slab); the host transposes/upcasts.
  - DMA is spread over three queues (x loads on the sync HWDGE ring, weight
    loads on the gpsimd SWDGE ring, stores on the scalar HWDGE ring) so
    loads never queue behind stores.
  - Weight prologue is a ladder over o-quarters: quarter 0 quantizes first,
    then the first BL_EARLY slabs' quarter-q matmuls run on the PE while
    quarter q+1's scale/quantization fills the DVE gaps.
"""

import os

import numpy as np

B, S, D_IN, D_OUT = 4, 2048, 2048, 8192
T = B * S  # 8192 tokens
N_CORES = 8
TH, OQN = 2, 4  # grid: token halves x out quarters
T_C = T // TH  # 4096 tokens per core
O_SHARD = D_OUT // OQN  # 2048 out features per core
EPS = 1e-05

P = 128
KC = D_IN // P  # 16 contraction chunks
T_SLAB = 512
N_SLABS = T_C // T_SLAB  # 8
N_OT = O_SHARD // P  # 16 o-tiles
OQ = 512  # o-columns per quant quarter
NQ = O_SHARD // OQ  # 4

NF8 = int(os.environ.get("BL_FP8_KC", "16"))  # k-chunks through fp8 DoubleRow
assert 0 <= NF8 <= KC and NF8 % 2 == 0
NB16 = KC - NF8
NPAIR = NF8 // 2
EARLY = int(os.environ.get("BL_EARLY", "5"))
EARLY = max(1, min(EARLY, N_SLABS))

_CACHE = {}


def _build_program():
    import concourse.bass as bass
    import concourse.tile as tile
    from concourse import bacc, mybir
    from concourse.masks import make_identity

    f32 = mybir.dt.float32
    bf16 = mybir.dt.bfloat16
    fp8 = mybir.dt.float8e4
    DR = mybir.MatmulPerfMode.DoubleRow

    nc = bacc.Bacc(
        "TRN2",
        target_bir_lowering=False,
        debug=False,
        num_devices=N_CORES,
    )

    xT = nc.dram_tensor("xT", [D_IN, T_C], f32, kind="ExternalInput")
    w = nc.dram_tensor("w", [O_SHARD, D_IN], f32, kind="ExternalInput")
    wT = nc.dram_tensor("wT", [D_IN, O_SHARD], f32, kind="ExternalInput")
    out = nc.dram_tensor("out", [O_SHARD, T_C], bf16, kind="ExternalOutput")

    xT3 = xT.ap().rearrange("(c p) t -> p c t", p=P)  # [128, 16, 4096]
    wT3 = wT.ap().rearrange("(c p) o -> p c o", p=P)  # [128, 16, 2048]
    out3 = out.ap().rearrange("(ot p) t -> p ot t", p=P)  # [128, 16, 4096]

    with tile.TileContext(nc) as tc:
        const_pool = tc.alloc_tile_pool(name="const", bufs=1)
        wq8_pool = tc.alloc_tile_pool(name="wq8", bufs=1)
        wq16_pool = tc.alloc_tile_pool(name="wq16", bufs=1)
        sb_pool = tc.alloc_tile_pool(name="thr", bufs=1)
        w_pool = tc.alloc_tile_pool(name="wstage", bufs=3)
        wt_pool = tc.alloc_tile_pool(name="wtstage", bufs=3)
        gl_pool = tc.alloc_tile_pool(name="gl", bufs=2)
        st_pool = tc.alloc_tile_pool(name="stats", bufs=4)
        psum_pro = tc.alloc_tile_pool(name="psum_pro", bufs=1, space="PSUM")
        xb_pool = tc.alloc_tile_pool(name="xb", bufs=EARLY + 1)
        xf_pool = tc.alloc_tile_pool(name="xf", bufs=3)
        out_pool = tc.alloc_tile_pool(name="osb", bufs=2)
        outh_pool = tc.alloc_tile_pool(name="osbh", bufs=2)
        psum_mm = tc.alloc_tile_pool(name="psum_mm", bufs=3, space="PSUM")
        ctx_pools = [const_pool, wq8_pool, wq16_pool, sb_pool, w_pool, wt_pool,
                     gl_pool, st_pool, psum_pro, xb_pool, xf_pool, out_pool,
                     outh_pool, psum_mm]

        ident_f32 = const_pool.tile([P, P], f32)
        make_identity(nc, ident_f32)
        ones_f32 = const_pool.tile([P, P], f32)
        nc.vector.memset(ones_f32[:], 1.0)

        # resident: ternary weights in [k, o] matmul orientation (fp8 chunks
        # + bf16 tail chunks), per-o-tile scale columns, and the +-threshold
        # broadcast across partitions (for quantization compares)
        wqT8 = wq8_pool.tile([P, NF8, O_SHARD], fp8, name="wqT8")
        wqT16 = (wq16_pool.tile([P, NB16, O_SHARD], bf16, name="wqT16")
                 if NB16 else None)
        scales = sb_pool.tile([P, N_OT], f32, tag="scales", name="scales")
        thrB = sb_pool.tile([P, O_SHARD], f32, tag="thrB", name="thrB")
        nthrB = sb_pool.tile([P, O_SHARD], f32, tag="nthrB", name="nthrB")

        def prologue_otile(ot):
            """Bit-exact per-row scale for o-tile `ot` + threshold broadcast."""
            wf = w_pool.tile([P, D_IN], f32, name="wf")
            eng = nc.gpsimd if ot < 4 else nc.scalar
            eng.dma_start(wf[:], w[bass.ts(ot, P), :])

            # blocked-512 two-stage reduce: bit-exact match with the
            # reference's lowered jnp.mean
            ssum4 = st_pool.tile([P, 4], f32, tag="ssum4", name="ssum4")
            nc.vector.tensor_reduce(
                out=ssum4[:],
                in_=wf[:].rearrange("p (b k) -> p b k", k=512),
                op=mybir.AluOpType.add,
                axis=mybir.AxisListType.X,
                apply_absolute_value=True,
            )
            ssum = st_pool.tile([P, 1], f32, tag="ssum", name="ssum")
            nc.vector.tensor_reduce(
                out=ssum[:], in_=ssum4[:],
                op=mybir.AluOpType.add, axis=mybir.AxisListType.X,
            )
            nc.vector.tensor_scalar(
                scales[:, bass.ds(ot, 1)], ssum[:], 1.0 / D_IN, EPS,
                mybir.AluOpType.mult, mybir.AluOpType.max,
            )
            thr = st_pool.tile([P, 1], f32, tag="thr", name="thr")
            nc.vector.tensor_scalar_mul(thr[:], scales[:, bass.ds(ot, 1)], 0.5)

            # thrB[:, ot*128:+128] = thr broadcast over partitions
            # (ones.T @ diag(thr)), likewise -thr into nthrB
            for sign, dst in ((1.0, thrB), (-1.0, nthrB)):
                ds_t = st_pool.tile([P, P], f32, tag=f"diag{sign}", name="ds_t")
                nc.vector.tensor_scalar(
                    ds_t[:], ident_f32[:], thr[:], sign,
                    mybir.AluOpType.mult, mybir.AluOpType.mult,
                )
                bp = psum_pro.tile([P, P], f32, tag="bp", name="bp", bufs=2)
                nc.tensor.matmul(bp[:], ones_f32[:], ds_t[:],
                                 start=True, stop=True)
                nc.scalar.copy(out=dst[:, bass.ts(ot, P)], in_=bp[:])

        def quant_tile(q, kc):
            """Quantize wT chunk kc, o-quarter q, into wqT8/wqT16."""
            qc = bass.ds(q * OQ, OQ)
            wtf = wt_pool.tile([P, OQ], f32, name="wtf")
            nc.gpsimd.dma_start(wtf[:], wT3[:, kc, qc])
            g = gl_pool.tile([P, OQ], f32, tag="g", name="g")
            nc.vector.tensor_tensor(g[:], wtf[:], thrB[:, qc],
                                    mybir.AluOpType.is_gt)
            l = gl_pool.tile([P, OQ], f32, tag="l", name="l")
            nc.vector.tensor_tensor(l[:], wtf[:], nthrB[:, qc],
                                    mybir.AluOpType.is_lt)
            dst = wqT8[:, kc, qc] if kc < NF8 else wqT16[:, kc - NF8, qc]
            nc.vector.tensor_tensor(dst, g[:], l[:], mybir.AluOpType.subtract)
        xb_tiles = {}

        def load_slab(s):
            tsl = bass.ts(s, T_SLAB)
            xb8 = xb_pool.tile([P, NF8, T_SLAB], fp8, tag="xb8", name="xb8")
            xb16 = (xb_pool.tile([P, NB16, T_SLAB], bf16, tag="xb16",
                                 name="xb16") if NB16 else None)
            for quar in range(4):
                xf = xf_pool.tile([P, 4, T_SLAB], f32, name="xf")
                nc.sync.dma_start(xf[:], xT3[:, bass.ts(quar, 4), tsl])
                base = quar * 4
                n8 = max(0, min(4, NF8 - base))
                if n8:
                    nc.scalar.copy(out=xb8[:, bass.ds(base, n8), :],
                                   in_=xf[:, bass.ds(0, n8), :])
                if n8 < 4:
                    nc.scalar.copy(
                        out=xb16[:, bass.ds(base + n8 - NF8, 4 - n8), :],
                        in_=xf[:, bass.ds(n8, 4 - n8), :])
            xb_tiles[s] = (xb8, xb16)

        def mm_group(ot, s, ps, side):
            """All matmuls for o-tile `ot` x token slab `s` (512 tokens)."""
            xb8, xb16 = xb_tiles[s]
            otc = bass.ts(ot, P)
            dst = ps[:, bass.ds(side * T_SLAB, T_SLAB)]
            n_mm = NPAIR + NB16
            idx = 0
            for kp in range(NPAIR):
                nc.tensor.matmul(
                    dst,
                    wqT8[:, bass.ds(2 * kp, 2), otc],
                    xb8[:, bass.ds(2 * kp, 2), :],
                    start=(idx == 0),
                    stop=(idx == n_mm - 1),
                    perf_mode=DR,
                )
                idx += 1
            for j in range(NB16):
                nc.tensor.matmul(
                    dst,
                    wqT16[:, j, otc],
                    xb16[:, j, :],
                    start=(idx == 0),
                    stop=(idx == n_mm - 1),
                )
                idx += 1

        def ot_pair(ot, s, osb, slot):
            """Two o-tiles x one slab through one 2-bank psum tile."""
            ps = psum_mm.tile([P, 2 * T_SLAB], f32, tag="ps", name="ps")
            mm_group(ot, s, ps, 0)
            mm_group(ot + 1, s, ps, 1)
            for i in (0, 1):
                nc.scalar.mul(osb[:, slot + i, :],
                              ps[:, bass.ds(i * T_SLAB, T_SLAB)],
                              scales[:, bass.ds(ot + i, 1)])

        # ---------------- emission schedule -----------------------------
        load_slab(0)
        for ot in range(4):
            prologue_otile(ot)
        for kc in range(KC):
            quant_tile(0, kc)
        for s in range(1, EARLY):
            load_slab(s)

        # Ladder over o-quarters: run the early slabs' quarter-q groups
        # while quarter q+1's prologue fills the DVE gaps.
        phase_tasks = {
            0: ([lambda ot=ot: prologue_otile(ot) for ot in range(4, 8)]
                + [lambda kc=kc: quant_tile(1, kc) for kc in range(KC)]),
            1: ([lambda ot=ot: prologue_otile(ot) for ot in range(8, 12)]
                + [lambda kc=kc: quant_tile(2, kc) for kc in range(KC)]),
            2: ([lambda ot=ot: prologue_otile(ot) for ot in range(12, 16)]
                + [lambda kc=kc: quant_tile(3, kc) for kc in range(KC)]),
            3: [],
        }
        for q in range(NQ):
            tasks = phase_tasks[q]
            units = [(s, 4 * q + 2 * pi) for s in range(EARLY)
                     for pi in range(2)]
            per = (len(tasks) + len(units) - 1) // len(units)
            ci = 0
            osbs = {}
            for (s, ot) in units:
                for _ in range(per):
                    if ci < len(tasks):
                        tasks[ci]()
                        ci += 1
                if s not in osbs:
                    osbs[s] = outh_pool.tile([P, 4, T_SLAB], bf16,
                                             name="osbh")
                ot_pair(ot, s, osbs[s], ot - 4 * q)
                if ot % 4 == 2:  # second pair of the quarter for this slab
                    nc.scalar.dma_start(
                        out3[:, bass.ds(4 * q, 4), bass.ts(s, T_SLAB)],
                        osbs.pop(s)[:])
            while ci < len(tasks):
                tasks[ci]()
                ci += 1

        # steady state: all 16 o-tiles per slab, one batched 2 MB store
        for s in range(EARLY, N_SLABS):
            load_slab(s)
            osb = out_pool.tile([P, N_OT, T_SLAB], bf16, name="osb")
            for pi in range(N_OT // 2):
                ot_pair(2 * pi, s, osb, 2 * pi)
            nc.scalar.dma_start(out3[:, :, bass.ts(s, T_SLAB)], osb[:])

        for p in reversed(ctx_pools):
            p.release()

    nc.compile()
    return nc


def _get_program():
    if "nc" not in _CACHE:
        _CACHE["nc"] = _build_program()
    return _CACHE["nc"]


def _ensure_ntff_hook():
    """Provide antenv.axon_hooks if the image lacks it (profiling only)."""
    import sys
    import types

    try:
        from antenv.axon_hooks import get_axon_ntff_profile_hook  # noqa: F401
        return
    except ImportError:
        pass
    try:
        import antenv
        from trn_agent_boot.trn_boot import _ntff_profile_via_ctypes

        mod = types.ModuleType("antenv.axon_hooks")
        state = {"hook": _ntff_profile_via_ctypes("/opt/axon/libaxon_pjrt.so")}
        mod.get_axon_ntff_profile_hook = lambda: state["hook"]
        mod.set_axon_ntff_profile_hook = lambda h: state.__setitem__("hook", h)
        sys.modules["antenv.axon_hooks"] = mod
        antenv.axon_hooks = mod
    except Exception:
        pass


def kernel(x: np.ndarray, weight: np.ndarray) -> np.ndarray:
    from concourse.bass_utils import run_bass_kernel_spmd

    assert x.shape == (B, S, D_IN) and weight.shape == (D_OUT, D_IN)
    nc = _get_program()

    xT = np.ascontiguousarray(x.reshape(T, D_IN).T)  # [D_IN, T]
    in_maps = []
    for c in range(N_CORES):
        th, oq = divmod(c, OQN)
        w_shard = weight[oq * O_SHARD:(oq + 1) * O_SHARD]
        in_maps.append({
            "xT": np.ascontiguousarray(xT[:, th * T_C:(th + 1) * T_C]),
            "w": w_shard,
            "wT": np.ascontiguousarray(w_shard.T),
        })

    trace = os.environ.get("BL_TRACE", "0") == "1"
    if trace:
        _ensure_ntff_hook()
    res = run_bass_kernel_spmd(nc, in_maps, list(range(N_CORES)), trace=trace)
    _CACHE["last_results"] = res

    fullT = np.empty((D_OUT, T), dtype=np.float32)  # [o, t]
    for c in range(N_CORES):
        th, oq = divmod(c, OQN)
        part = np.asarray(res.results[c]["out"]).astype(np.float32)
        fullT[oq * O_SHARD:(oq + 1) * O_SHARD, th * T_C:(th + 1) * T_C] = part
    return np.ascontiguousarray(fullT.T.reshape(B, S, D_OUT))


# revision 28
# speedup vs baseline: 1.0953x; 1.0953x over previous
"""BitLinear (ternary-quantized linear) Trainium2 kernel, v3.

Computes: scale = clip(mean(|w|, axis=1), 1e-5);  w_q = clip(round(w/scale), -1, 1)
          out = x @ (w_q * scale).T
for x [4, 2048, 2048] f32, w [8192, 2048] f32, out [4, 2048, 8192] f32.

Strategy (8 NeuronCores in a 2x4 token-half x out-quarter grid):
  - Core (th, oq) computes out features [oq*2048, (oq+1)*2048) for tokens
    [th*4096, (th+1)*4096): x traffic halves vs pure output sharding; total
    per-core HBM traffic is 32(x) + 32(w both layouts) + 16(out bf16) = 80 MB.
  - Host feeds x transposed [d_in, tokens] and the w shard in BOTH layouts
    ([o, k] for the bit-exact per-row scale reduction on the DVE, [k, o] for
    quantization in matmul orientation) - layout choices only; all
    arithmetic happens on device.
  - Per-row scale uses the blocked-512 two-stage reduction (bit-exact match
    with the reference's jnp.mean lowering); w_q = (w > s/2) - (w < -s/2)
    equals clip(round(w/scale), -1, 1) exactly. Ternary w_q is exact in
    fp8e4 and bf16.
  - The contraction (2048 = 16 chunks of 128) is split: the first BL_FP8_KC
    (default 14) chunks run as fp8e4 DoubleRow matmuls (2 chunks/matmul, 2x
    PE rate), the rest as bf16 matmuls. x is cast f32->fp8e4/bf16 on the
    scalar engine per 512-token slab. Measured end-to-end error ~1.9e-2
    scale-relative (gate 2e-2).
  - Matmuls run with w_q stationary and x moving, so PSUM is [o, tokens]
    and the per-o scale is per-PARTITION: the epilogue is a scalar-engine
    activation copy (fast PSUM reads), keeping the vector engine free for
    quantization. Output is stored bf16 in [o, t] layout (contiguous rows,
    one 2 MB store per slab); the host transposes/upcasts.
  - DMA is spread over three queues (x loads on the sync HWDGE ring, weight
    loads on the gpsimd SWDGE ring, stores on the scalar HWDGE ring) so
    loads never queue behind stores.
  - Weight prologue is a ladder over o-quarters: quarter 0 quantizes first,
    then the first BL_EARLY slabs' quarter-q matmuls run on the PE while
    quarter q+1's scale/quantization fills the DVE gaps.
"""

import os

import numpy as np

B, S, D_IN, D_OUT = 4, 2048, 2048, 8192
T = B * S  # 8192 tokens
N_CORES = 8
TH, OQN = 2, 4  # grid: token halves x out quarters
T_C = T // TH  # 4096 tokens per core
O_SHARD = D_OUT // OQN  # 2048 out features per core
EPS = 1e-05

P = 128
KC = D_IN // P  # 16 contraction chunks
T_SLAB = 512
N_SLABS = T_C // T_SLAB  # 8
N_OT = O_SHARD // P  # 16 o-tiles
OQ = 512  # o-columns per quant quarter
NQ = O_SHARD // OQ  # 4

NF8 = int(os.environ.get("BL_FP8_KC", "16"))  # k-chunks through fp8 DoubleRow
assert 0 <= NF8 <= KC and NF8 % 2 == 0
NB16 = KC - NF8
NPAIR = NF8 // 2
EARLY = int(os.environ.get("BL_EARLY", "6"))
EARLY = max(1, min(EARLY, N_SLABS))

_CACHE = {}


def _build_program():
    import concourse.bass as bass
    import concourse.tile as tile
    from concourse import bacc, mybir
    from concourse.masks import make_identity

    f32 = mybir.dt.float32
    bf16 = mybir.dt.bfloat16
    fp8 = mybir.dt.float8e4
    DR = mybir.MatmulPerfMode.DoubleRow

    nc = bacc.Bacc(
        "TRN2",
        target_bir_lowering=False,
        debug=False,
        num_devices=N_CORES,
    )

    xT = nc.dram_tensor("xT", [D_IN, T_C], f32, kind="ExternalInput")
    w = nc.dram_tensor("w", [O_SHARD, D_IN], f32, kind="ExternalInput")
    wT = nc.dram_tensor("wT", [D_IN, O_SHARD], f32, kind="ExternalInput")
    out = nc.dram_tensor("out", [O_SHARD, T_C], bf16, kind="ExternalOutput")

    xT3 = xT.ap().rearrange("(c p) t -> p c t", p=P)  # [128, 16, 4096]
    wT3 = wT.ap().rearrange("(c p) o -> p c o", p=P)  # [128, 16, 2048]
    out3 = out.ap().rearrange("(ot p) t -> p ot t", p=P)  # [128, 16, 4096]

    with tile.TileContext(nc) as tc:
        const_pool = tc.alloc_tile_pool(name="const", bufs=1)
        wq8_pool = tc.alloc_tile_pool(name="wq8", bufs=1)
        wq16_pool = tc.alloc_tile_pool(name="wq16", bufs=1)
        sb_pool = tc.alloc_tile_pool(name="thr", bufs=1)
        w_pool = tc.alloc_tile_pool(name="wstage", bufs=2)
        wt_pool = tc.alloc_tile_pool(name="wtstage", bufs=3)
        gl_pool = tc.alloc_tile_pool(name="gl", bufs=2)
        st_pool = tc.alloc_tile_pool(name="stats", bufs=4)
        psum_pro = tc.alloc_tile_pool(name="psum_pro", bufs=1, space="PSUM")
        xb_pool = tc.alloc_tile_pool(name="xb", bufs=EARLY + 1)
        xf_pool = tc.alloc_tile_pool(name="xf", bufs=3)
        out_pool = tc.alloc_tile_pool(name="osb", bufs=2)
        outh_pool = tc.alloc_tile_pool(name="osbh", bufs=2)
        psum_mm = tc.alloc_tile_pool(name="psum_mm", bufs=3, space="PSUM")
        ctx_pools = [const_pool, wq8_pool, wq16_pool, sb_pool, w_pool, wt_pool,
                     gl_pool, st_pool, psum_pro, xb_pool, xf_pool, out_pool,
                     outh_pool, psum_mm]

        ident_f32 = const_pool.tile([P, P], f32)
        make_identity(nc, ident_f32)
        ones_f32 = const_pool.tile([P, P], f32)
        nc.vector.memset(ones_f32[:], 1.0)

        # resident: ternary weights in [k, o] matmul orientation (fp8 chunks
        # + bf16 tail chunks), per-o-tile scale columns, and the +-threshold
        # broadcast across partitions (for quantization compares)
        wqT8 = wq8_pool.tile([P, NF8, O_SHARD], fp8, name="wqT8")
        wqT16 = (wq16_pool.tile([P, NB16, O_SHARD], bf16, name="wqT16")
                 if NB16 else None)
        scales = sb_pool.tile([P, N_OT], f32, tag="scales", name="scales")
        thrB = sb_pool.tile([P, O_SHARD], f32, tag="thrB", name="thrB")
        nthrB = sb_pool.tile([P, O_SHARD], f32, tag="nthrB", name="nthrB")

        def prologue_otile(ot):
            """Bit-exact per-row scale for o-tile `ot` + threshold broadcast."""
            wf = w_pool.tile([P, D_IN], f32, name="wf")
            eng = nc.gpsimd if ot < 4 else nc.scalar
            eng.dma_start(wf[:], w[bass.ts(ot, P), :])

            # blocked-512 two-stage reduce: bit-exact match with the
            # reference's lowered jnp.mean
            ssum4 = st_pool.tile([P, 4], f32, tag="ssum4", name="ssum4")
            nc.vector.tensor_reduce(
                out=ssum4[:],
                in_=wf[:].rearrange("p (b k) -> p b k", k=512),
                op=mybir.AluOpType.add,
                axis=mybir.AxisListType.X,
                apply_absolute_value=True,
            )
            ssum = st_pool.tile([P, 1], f32, tag="ssum", name="ssum")
            nc.vector.tensor_reduce(
                out=ssum[:], in_=ssum4[:],
                op=mybir.AluOpType.add, axis=mybir.AxisListType.X,
            )
            nc.vector.tensor_scalar(
                scales[:, bass.ds(ot, 1)], ssum[:], 1.0 / D_IN, EPS,
                mybir.AluOpType.mult, mybir.AluOpType.max,
            )
            thr = st_pool.tile([P, 1], f32, tag="thr", name="thr")
            nc.vector.tensor_scalar_mul(thr[:], scales[:, bass.ds(ot, 1)], 0.5)

            # thrB[:, ot*128:+128] = thr broadcast over partitions
            # (ones.T @ diag(thr)), likewise -thr into nthrB
            for sign, dst in ((1.0, thrB), (-1.0, nthrB)):
                ds_t = st_pool.tile([P, P], f32, tag=f"diag{sign}", name="ds_t")
                nc.vector.tensor_scalar(
                    ds_t[:], ident_f32[:], thr[:], sign,
                    mybir.AluOpType.mult, mybir.AluOpType.mult,
                )
                bp = psum_pro.tile([P, P], f32, tag="bp", name="bp", bufs=2)
                nc.tensor.matmul(bp[:], ones_f32[:], ds_t[:],
                                 start=True, stop=True)
                nc.scalar.copy(out=dst[:, bass.ts(ot, P)], in_=bp[:])

        def quant_tile(q, kc):
            """Quantize wT chunk kc, o-quarter q, into wqT8/wqT16."""
            qc = bass.ds(q * OQ, OQ)
            wtf = wt_pool.tile([P, OQ], f32, name="wtf")
            nc.gpsimd.dma_start(wtf[:], wT3[:, kc, qc])
            g = gl_pool.tile([P, OQ], f32, tag="g", name="g")
            nc.vector.tensor_tensor(g[:], wtf[:], thrB[:, qc],
                                    mybir.AluOpType.is_gt)
            l = gl_pool.tile([P, OQ], f32, tag="l", name="l")
            nc.vector.tensor_tensor(l[:], wtf[:], nthrB[:, qc],
                                    mybir.AluOpType.is_lt)
            dst = wqT8[:, kc, qc] if kc < NF8 else wqT16[:, kc - NF8, qc]
            nc.vector.tensor_tensor(dst, g[:], l[:], mybir.AluOpType.subtract)
        xb_tiles = {}

        def load_slab(s):
            tsl = bass.ts(s, T_SLAB)
            xb8 = xb_pool.tile([P, NF8, T_SLAB], fp8, tag="xb8", name="xb8")
            xb16 = (xb_pool.tile([P, NB16, T_SLAB], bf16, tag="xb16",
                                 name="xb16") if NB16 else None)
            for quar in range(4):
                xf = xf_pool.tile([P, 4, T_SLAB], f32, name="xf")
                nc.sync.dma_start(xf[:], xT3[:, bass.ts(quar, 4), tsl])
                base = quar * 4
                n8 = max(0, min(4, NF8 - base))
                if n8:
                    nc.scalar.copy(out=xb8[:, bass.ds(base, n8), :],
                                   in_=xf[:, bass.ds(0, n8), :])
                if n8 < 4:
                    nc.scalar.copy(
                        out=xb16[:, bass.ds(base + n8 - NF8, 4 - n8), :],
                        in_=xf[:, bass.ds(n8, 4 - n8), :])
            xb_tiles[s] = (xb8, xb16)

        def mm_group(ot, s, ps, side):
            """All matmuls for o-tile `ot` x token slab `s` (512 tokens)."""
            xb8, xb16 = xb_tiles[s]
            otc = bass.ts(ot, P)
            dst = ps[:, bass.ds(side * T_SLAB, T_SLAB)]
            n_mm = NPAIR + NB16
            idx = 0
            for kp in range(NPAIR):
                nc.tensor.matmul(
                    dst,
                    wqT8[:, bass.ds(2 * kp, 2), otc],
                    xb8[:, bass.ds(2 * kp, 2), :],
                    start=(idx == 0),
                    stop=(idx == n_mm - 1),
                    perf_mode=DR,
                )
                idx += 1
            for j in range(NB16):
                nc.tensor.matmul(
                    dst,
                    wqT16[:, j, otc],
                    xb16[:, j, :],
                    start=(idx == 0),
                    stop=(idx == n_mm - 1),
                )
                idx += 1

        def ot_pair(ot, s, osb, slot):
            """Two o-tiles x one slab through one 2-bank psum tile."""
            ps = psum_mm.tile([P, 2 * T_SLAB], f32, tag="ps", name="ps")
            mm_group(ot, s, ps, 0)
            mm_group(ot + 1, s, ps, 1)
            for i in (0, 1):
                nc.scalar.mul(osb[:, slot + i, :],
                              ps[:, bass.ds(i * T_SLAB, T_SLAB)],
                              scales[:, bass.ds(ot + i, 1)])

        # ---------------- emission schedule -----------------------------
        load_slab(0)
        for ot in range(4):
            prologue_otile(ot)
        for kc in range(KC):
            quant_tile(0, kc)
        for s in range(1, EARLY):
            load_slab(s)

        # Ladder over o-quarters: run the early slabs' quarter-q groups
        # while quarter q+1's prologue fills the DVE gaps.
        # each quarter's scale prologue runs one phase ahead of its quant
        # pass, so quant q+1 can start the moment phase q begins
        phase_tasks = {
            0: ([lambda ot=ot: prologue_otile(ot) for ot in range(4, 12)]
                + [lambda kc=kc: quant_tile(1, kc) for kc in range(KC)]),
            1: ([lambda ot=ot: prologue_otile(ot) for ot in range(12, 16)]
                + [lambda kc=kc: quant_tile(2, kc) for kc in range(KC)]),
            2: [lambda kc=kc: quant_tile(3, kc) for kc in range(KC)],
            3: [],
        }
        for q in range(NQ):
            tasks = phase_tasks[q]
            units = [(s, 4 * q + 2 * pi) for s in range(EARLY)
                     for pi in range(2)]
            per = (3 * len(tasks) + 2 * len(units) - 1) // (2 * len(units))
            ci = 0
            osbs = {}
            for (s, ot) in units:
                for _ in range(per):
                    if ci < len(tasks):
                        tasks[ci]()
                        ci += 1
                if s not in osbs:
                    osbs[s] = outh_pool.tile([P, 4, T_SLAB], bf16,
                                             name="osbh")
                ot_pair(ot, s, osbs[s], ot - 4 * q)
                if ot % 4 == 2:  # second pair of the quarter for this slab
                    nc.scalar.dma_start(
                        out3[:, bass.ds(4 * q, 4), bass.ts(s, T_SLAB)],
                        osbs.pop(s)[:])
            while ci < len(tasks):
                tasks[ci]()
                ci += 1

        # steady state: all 16 o-tiles per slab, one batched 2 MB store
        for s in range(EARLY, N_SLABS):
            load_slab(s)
            osb = out_pool.tile([P, N_OT, T_SLAB], bf16, name="osb")
            for pi in range(N_OT // 2):
                ot_pair(2 * pi, s, osb, 2 * pi)
            nc.scalar.dma_start(out3[:, :, bass.ts(s, T_SLAB)], osb[:])

        for p in reversed(ctx_pools):
            p.release()

    nc.compile()
    return nc


def _get_program():
    if "nc" not in _CACHE:
        _CACHE["nc"] = _build_program()
    return _CACHE["nc"]


def _ensure_ntff_hook():
    """Provide antenv.axon_hooks if the image lacks it (profiling only)."""
    import sys
    import types

    try:
        from antenv.axon_hooks import get_axon_ntff_profile_hook  # noqa: F401
        return
    except ImportError:
        pass
    try:
        import antenv
        from trn_agent_boot.trn_boot import _ntff_profile_via_ctypes

        mod = types.ModuleType("antenv.axon_hooks")
        state = {"hook": _ntff_profile_via_ctypes("/opt/axon/libaxon_pjrt.so")}
        mod.get_axon_ntff_profile_hook = lambda: state["hook"]
        mod.set_axon_ntff_profile_hook = lambda h: state.__setitem__("hook", h)
        sys.modules["antenv.axon_hooks"] = mod
        antenv.axon_hooks = mod
    except Exception:
        pass


def kernel(x: np.ndarray, weight: np.ndarray) -> np.ndarray:
    from concourse.bass_utils import run_bass_kernel_spmd

    assert x.shape == (B, S, D_IN) and weight.shape == (D_OUT, D_IN)
    nc = _get_program()

    xT = np.ascontiguousarray(x.reshape(T, D_IN).T)  # [D_IN, T]
    in_maps = []
    for c in range(N_CORES):
        th, oq = divmod(c, OQN)
        w_shard = weight[oq * O_SHARD:(oq + 1) * O_SHARD]
        in_maps.append({
            "xT": np.ascontiguousarray(xT[:, th * T_C:(th + 1) * T_C]),
            "w": w_shard,
            "wT": np.ascontiguousarray(w_shard.T),
        })

    trace = os.environ.get("BL_TRACE", "0") == "1"
    if trace:
        _ensure_ntff_hook()
    res = run_bass_kernel_spmd(nc, in_maps, list(range(N_CORES)), trace=trace)
    _CACHE["last_results"] = res

    fullT = np.empty((D_OUT, T), dtype=np.float32)  # [o, t]
    for c in range(N_CORES):
        th, oq = divmod(c, OQN)
        part = np.asarray(res.results[c]["out"]).astype(np.float32)
        fullT[oq * O_SHARD:(oq + 1) * O_SHARD, th * T_C:(th + 1) * T_C] = part
    return np.ascontiguousarray(fullT.T.reshape(B, S, D_OUT))


# revision 29
# speedup vs baseline: 1.1132x; 1.0163x over previous
"""BitLinear (ternary-quantized linear) Trainium2 kernel, v3.

Computes: scale = clip(mean(|w|, axis=1), 1e-5);  w_q = clip(round(w/scale), -1, 1)
          out = x @ (w_q * scale).T
for x [4, 2048, 2048] f32, w [8192, 2048] f32, out [4, 2048, 8192] f32.

Strategy (8 NeuronCores in a 2x4 token-half x out-quarter grid):
  - Core (th, oq) computes out features [oq*2048, (oq+1)*2048) for tokens
    [th*4096, (th+1)*4096): x traffic halves vs pure output sharding; total
    per-core HBM traffic is 32(x) + 32(w both layouts) + 16(out bf16) = 80 MB.
  - Host feeds x transposed [d_in, tokens] and the w shard in BOTH layouts
    ([o, k] for the bit-exact per-row scale reduction on the DVE, [k, o] for
    quantization in matmul orientation) - layout choices only; all
    arithmetic happens on device.
  - Per-row scale uses the blocked-512 two-stage reduction (bit-exact match
    with the reference's jnp.mean lowering); w_q = (w > s/2) - (w < -s/2)
    equals clip(round(w/scale), -1, 1) exactly. Ternary w_q is exact in
    fp8e4 and bf16.
  - The contraction (2048 = 16 chunks of 128) is split: the first BL_FP8_KC
    (default 14) chunks run as fp8e4 DoubleRow matmuls (2 chunks/matmul, 2x
    PE rate), the rest as bf16 matmuls. x is cast f32->fp8e4/bf16 on the
    scalar engine per 512-token slab. Measured end-to-end error ~1.9e-2
    scale-relative (gate 2e-2).
  - Matmuls run with w_q stationary and x moving, so PSUM is [o, tokens]
    and the per-o scale is per-PARTITION: the epilogue is a scalar-engine
    activation copy (fast PSUM reads), keeping the vector engine free for
    quantization. Output is stored bf16 in [o, t] layout (contiguous rows,
    one 2 MB store per slab); the host transposes/upcasts.
  - DMA is spread over three queues (x loads on the sync HWDGE ring, weight
    loads on the gpsimd SWDGE ring, stores on the scalar HWDGE ring) so
    loads never queue behind stores.
  - Weight prologue is a ladder over o-quarters: quarter 0 quantizes first,
    then the first BL_EARLY slabs' quarter-q matmuls run on the PE while
    quarter q+1's scale/quantization fills the DVE gaps.
"""

import os

import numpy as np

B, S, D_IN, D_OUT = 4, 2048, 2048, 8192
T = B * S  # 8192 tokens
N_CORES = 8
TH, OQN = 2, 4  # grid: token halves x out quarters
T_C = T // TH  # 4096 tokens per core
O_SHARD = D_OUT // OQN  # 2048 out features per core
EPS = 1e-05

P = 128
KC = D_IN // P  # 16 contraction chunks
T_SLAB = 512
N_SLABS = T_C // T_SLAB  # 8
N_OT = O_SHARD // P  # 16 o-tiles
OQ = 512  # o-columns per quant quarter
NQ = O_SHARD // OQ  # 4

NF8 = int(os.environ.get("BL_FP8_KC", "16"))  # k-chunks through fp8 DoubleRow
assert 0 <= NF8 <= KC and NF8 % 2 == 0
NB16 = KC - NF8
NPAIR = NF8 // 2
EARLY = int(os.environ.get("BL_EARLY", "6"))
EARLY = max(1, min(EARLY, N_SLABS))

_CACHE = {}


def _build_program():
    import concourse.bass as bass
    import concourse.tile as tile
    from concourse import bacc, mybir
    from concourse.masks import make_identity

    f32 = mybir.dt.float32
    bf16 = mybir.dt.bfloat16
    fp8 = mybir.dt.float8e4
    DR = mybir.MatmulPerfMode.DoubleRow

    nc = bacc.Bacc(
        "TRN2",
        target_bir_lowering=False,
        debug=False,
        num_devices=N_CORES,
    )

    xT = nc.dram_tensor("xT", [D_IN, T_C], f32, kind="ExternalInput")
    w = nc.dram_tensor("w", [O_SHARD, D_IN], f32, kind="ExternalInput")
    wT = nc.dram_tensor("wT", [D_IN, O_SHARD], f32, kind="ExternalInput")
    out = nc.dram_tensor("out", [O_SHARD, T_C], bf16, kind="ExternalOutput")

    xT3 = xT.ap().rearrange("(c p) t -> p c t", p=P)  # [128, 16, 4096]
    wT3 = wT.ap().rearrange("(c p) o -> p c o", p=P)  # [128, 16, 2048]
    out3 = out.ap().rearrange("(ot p) t -> p ot t", p=P)  # [128, 16, 4096]

    with tile.TileContext(nc) as tc:
        const_pool = tc.alloc_tile_pool(name="const", bufs=1)
        wq8_pool = tc.alloc_tile_pool(name="wq8", bufs=1)
        wq16_pool = tc.alloc_tile_pool(name="wq16", bufs=1)
        sb_pool = tc.alloc_tile_pool(name="thr", bufs=1)
        w_pool = tc.alloc_tile_pool(name="wstage", bufs=2)
        wt_pool = tc.alloc_tile_pool(name="wtstage", bufs=3)
        gl_pool = tc.alloc_tile_pool(name="gl", bufs=2)
        st_pool = tc.alloc_tile_pool(name="stats", bufs=4)
        psum_pro = tc.alloc_tile_pool(name="psum_pro", bufs=1, space="PSUM")
        xb_pool = tc.alloc_tile_pool(name="xb", bufs=EARLY + 1)
        xf_pool = tc.alloc_tile_pool(name="xf", bufs=3)
        out_pool = tc.alloc_tile_pool(name="osb", bufs=2)
        outh_pool = tc.alloc_tile_pool(name="osbh", bufs=2)
        psum_mm = tc.alloc_tile_pool(name="psum_mm", bufs=3, space="PSUM")
        ctx_pools = [const_pool, wq8_pool, wq16_pool, sb_pool, w_pool, wt_pool,
                     gl_pool, st_pool, psum_pro, xb_pool, xf_pool, out_pool,
                     outh_pool, psum_mm]

        ident_f32 = const_pool.tile([P, P], f32)
        make_identity(nc, ident_f32)
        ones_f32 = const_pool.tile([P, P], f32)
        nc.vector.memset(ones_f32[:], 1.0)

        # resident: ternary weights in [k, o] matmul orientation (fp8 chunks
        # + bf16 tail chunks), per-o-tile scale columns, and the +-threshold
        # broadcast across partitions (for quantization compares)
        wqT8 = wq8_pool.tile([P, NF8, O_SHARD], fp8, name="wqT8")
        wqT16 = (wq16_pool.tile([P, NB16, O_SHARD], bf16, name="wqT16")
                 if NB16 else None)
        scales = sb_pool.tile([P, N_OT], f32, tag="scales", name="scales")
        thrB = sb_pool.tile([P, O_SHARD], f32, tag="thrB", name="thrB")
        nthrB = sb_pool.tile([P, O_SHARD], f32, tag="nthrB", name="nthrB")

        def prologue_otile(ot):
            """Bit-exact per-row scale for o-tile `ot` + threshold broadcast."""
            wf = w_pool.tile([P, D_IN], f32, name="wf")
            eng = nc.gpsimd if ot < 4 else nc.scalar
            eng.dma_start(wf[:], w[bass.ts(ot, P), :])

            # blocked-512 two-stage reduce: bit-exact match with the
            # reference's lowered jnp.mean
            ssum4 = st_pool.tile([P, 4], f32, tag="ssum4", name="ssum4")
            nc.vector.tensor_reduce(
                out=ssum4[:],
                in_=wf[:].rearrange("p (b k) -> p b k", k=512),
                op=mybir.AluOpType.add,
                axis=mybir.AxisListType.X,
                apply_absolute_value=True,
            )
            ssum = st_pool.tile([P, 1], f32, tag="ssum", name="ssum")
            nc.vector.tensor_reduce(
                out=ssum[:], in_=ssum4[:],
                op=mybir.AluOpType.add, axis=mybir.AxisListType.X,
            )
            nc.vector.tensor_scalar(
                scales[:, bass.ds(ot, 1)], ssum[:], 1.0 / D_IN, EPS,
                mybir.AluOpType.mult, mybir.AluOpType.max,
            )
            thr = st_pool.tile([P, 1], f32, tag="thr", name="thr")
            nc.vector.tensor_scalar_mul(thr[:], scales[:, bass.ds(ot, 1)], 0.5)

            # thrB[:, ot*128:+128] = thr broadcast over partitions
            # (ones.T @ diag(thr)), likewise -thr into nthrB
            for sign, dst in ((1.0, thrB), (-1.0, nthrB)):
                ds_t = st_pool.tile([P, P], f32, tag=f"diag{sign}", name="ds_t")
                nc.vector.tensor_scalar(
                    ds_t[:], ident_f32[:], thr[:], sign,
                    mybir.AluOpType.mult, mybir.AluOpType.mult,
                )
                bp = psum_pro.tile([P, P], f32, tag="bp", name="bp", bufs=2)
                nc.tensor.matmul(bp[:], ones_f32[:], ds_t[:],
                                 start=True, stop=True)
                nc.scalar.copy(out=dst[:, bass.ts(ot, P)], in_=bp[:])

        def quant_tile(q, kc):
            """Quantize wT chunk kc, o-quarter q, into wqT8/wqT16."""
            qc = bass.ds(q * OQ, OQ)
            wtf = wt_pool.tile([P, OQ], f32, name="wtf")
            nc.gpsimd.dma_start(wtf[:], wT3[:, kc, qc])
            g = gl_pool.tile([P, OQ], f32, tag="g", name="g")
            nc.vector.tensor_tensor(g[:], wtf[:], thrB[:, qc],
                                    mybir.AluOpType.is_gt)
            l = gl_pool.tile([P, OQ], f32, tag="l", name="l")
            nc.vector.tensor_tensor(l[:], wtf[:], nthrB[:, qc],
                                    mybir.AluOpType.is_lt)
            dst = wqT8[:, kc, qc] if kc < NF8 else wqT16[:, kc - NF8, qc]
            nc.vector.tensor_tensor(dst, g[:], l[:], mybir.AluOpType.subtract)
        xb_tiles = {}

        def load_slab(s):
            tsl = bass.ts(s, T_SLAB)
            xb8 = xb_pool.tile([P, NF8, T_SLAB], fp8, tag="xb8", name="xb8")
            xb16 = (xb_pool.tile([P, NB16, T_SLAB], bf16, tag="xb16",
                                 name="xb16") if NB16 else None)
            for quar in range(4):
                xf = xf_pool.tile([P, 4, T_SLAB], f32, name="xf")
                nc.sync.dma_start(xf[:], xT3[:, bass.ts(quar, 4), tsl])
                base = quar * 4
                n8 = max(0, min(4, NF8 - base))
                if n8:
                    nc.scalar.copy(out=xb8[:, bass.ds(base, n8), :],
                                   in_=xf[:, bass.ds(0, n8), :])
                if n8 < 4:
                    nc.scalar.copy(
                        out=xb16[:, bass.ds(base + n8 - NF8, 4 - n8), :],
                        in_=xf[:, bass.ds(n8, 4 - n8), :])
            xb_tiles[s] = (xb8, xb16)

        def mm_group(ot, s, ps, side):
            """All matmuls for o-tile `ot` x token slab `s` (512 tokens)."""
            xb8, xb16 = xb_tiles[s]
            otc = bass.ts(ot, P)
            dst = ps[:, bass.ds(side * T_SLAB, T_SLAB)]
            n_mm = NPAIR + NB16
            idx = 0
            for kp in range(NPAIR):
                nc.tensor.matmul(
                    dst,
                    wqT8[:, bass.ds(2 * kp, 2), otc],
                    xb8[:, bass.ds(2 * kp, 2), :],
                    start=(idx == 0),
                    stop=(idx == n_mm - 1),
                    perf_mode=DR,
                )
                idx += 1
            for j in range(NB16):
                nc.tensor.matmul(
                    dst,
                    wqT16[:, j, otc],
                    xb16[:, j, :],
                    start=(idx == 0),
                    stop=(idx == n_mm - 1),
                )
                idx += 1

        def ot_pair(ot, s, osb, slot):
            """Two o-tiles x one slab through one 2-bank psum tile."""
            ps = psum_mm.tile([P, 2 * T_SLAB], f32, tag="ps", name="ps")
            mm_group(ot, s, ps, 0)
            mm_group(ot + 1, s, ps, 1)
            for i in (0, 1):
                nc.scalar.mul(osb[:, slot + i, :],
                              ps[:, bass.ds(i * T_SLAB, T_SLAB)],
                              scales[:, bass.ds(ot + i, 1)])

        # ---------------- emission schedule -----------------------------
        load_slab(0)
        for ot in range(4):
            prologue_otile(ot)
        for kc in range(KC):
            quant_tile(0, kc)
        for s in range(1, EARLY):
            load_slab(s)

        # Ladder over o-quarters: run the early slabs' quarter-q groups
        # while quarter q+1's prologue fills the DVE gaps.
        # each quarter's scale prologue runs one phase ahead of its quant
        # pass, so quant q+1 can start the moment phase q begins
        phase_tasks = {
            0: ([lambda ot=ot: prologue_otile(ot) for ot in range(4, 12)]
                + [lambda kc=kc: quant_tile(1, kc) for kc in range(KC)]),
            1: ([lambda ot=ot: prologue_otile(ot) for ot in range(12, 16)]
                + [lambda kc=kc: quant_tile(2, kc) for kc in range(KC)]
                + [lambda kc=kc: quant_tile(3, kc) for kc in range(KC // 2)]),
            2: [lambda kc=kc: quant_tile(3, kc) for kc in range(KC // 2, KC)],
            3: [],
        }
        for q in range(NQ):
            tasks = phase_tasks[q]
            units = [(s, 4 * q + 2 * pi) for s in range(EARLY)
                     for pi in range(2)]
            per = (3 * len(tasks) + 2 * len(units) - 1) // (2 * len(units))
            ci = 0
            osbs = {}
            for (s, ot) in units:
                for _ in range(per):
                    if ci < len(tasks):
                        tasks[ci]()
                        ci += 1
                if s not in osbs:
                    osbs[s] = outh_pool.tile([P, 4, T_SLAB], bf16,
                                             name="osbh")
                ot_pair(ot, s, osbs[s], ot - 4 * q)
                if ot % 4 == 2:  # second pair of the quarter for this slab
                    nc.scalar.dma_start(
                        out3[:, bass.ds(4 * q, 4), bass.ts(s, T_SLAB)],
                        osbs.pop(s)[:])
            while ci < len(tasks):
                tasks[ci]()
                ci += 1

        # steady state: all 16 o-tiles per slab, one batched 2 MB store
        for s in range(EARLY, N_SLABS):
            load_slab(s)
            osb = out_pool.tile([P, N_OT, T_SLAB], bf16, name="osb")
            for pi in range(N_OT // 2):
                ot_pair(2 * pi, s, osb, 2 * pi)
            nc.scalar.dma_start(out3[:, :, bass.ts(s, T_SLAB)], osb[:])

        for p in reversed(ctx_pools):
            p.release()

    nc.compile()
    return nc


def _get_program():
    if "nc" not in _CACHE:
        _CACHE["nc"] = _build_program()
    return _CACHE["nc"]


def _ensure_ntff_hook():
    """Provide antenv.axon_hooks if the image lacks it (profiling only)."""
    import sys
    import types

    try:
        from antenv.axon_hooks import get_axon_ntff_profile_hook  # noqa: F401
        return
    except ImportError:
        pass
    try:
        import antenv
        from trn_agent_boot.trn_boot import _ntff_profile_via_ctypes

        mod = types.ModuleType("antenv.axon_hooks")
        state = {"hook": _ntff_profile_via_ctypes("/opt/axon/libaxon_pjrt.so")}
        mod.get_axon_ntff_profile_hook = lambda: state["hook"]
        mod.set_axon_ntff_profile_hook = lambda h: state.__setitem__("hook", h)
        sys.modules["antenv.axon_hooks"] = mod
        antenv.axon_hooks = mod
    except Exception:
        pass


def kernel(x: np.ndarray, weight: np.ndarray) -> np.ndarray:
    from concourse.bass_utils import run_bass_kernel_spmd

    assert x.shape == (B, S, D_IN) and weight.shape == (D_OUT, D_IN)
    nc = _get_program()

    xT = np.ascontiguousarray(x.reshape(T, D_IN).T)  # [D_IN, T]
    in_maps = []
    for c in range(N_CORES):
        th, oq = divmod(c, OQN)
        w_shard = weight[oq * O_SHARD:(oq + 1) * O_SHARD]
        in_maps.append({
            "xT": np.ascontiguousarray(xT[:, th * T_C:(th + 1) * T_C]),
            "w": w_shard,
            "wT": np.ascontiguousarray(w_shard.T),
        })

    trace = os.environ.get("BL_TRACE", "0") == "1"
    if trace:
        _ensure_ntff_hook()
    res = run_bass_kernel_spmd(nc, in_maps, list(range(N_CORES)), trace=trace)
    _CACHE["last_results"] = res

    fullT = np.empty((D_OUT, T), dtype=np.float32)  # [o, t]
    for c in range(N_CORES):
        th, oq = divmod(c, OQN)
        part = np.asarray(res.results[c]["out"]).astype(np.float32)
        fullT[oq * O_SHARD:(oq + 1) * O_SHARD, th * T_C:(th + 1) * T_C] = part
    return np.ascontiguousarray(fullT.T.reshape(B, S, D_OUT))


# revision 30
# speedup vs baseline: 1.1280x; 1.0133x over previous
"""BitLinear (ternary-quantized linear) Trainium2 kernel, v3.

Computes: scale = clip(mean(|w|, axis=1), 1e-5);  w_q = clip(round(w/scale), -1, 1)
          out = x @ (w_q * scale).T
for x [4, 2048, 2048] f32, w [8192, 2048] f32, out [4, 2048, 8192] f32.

Strategy (8 NeuronCores in a 2x4 token-half x out-quarter grid):
  - Core (th, oq) computes out features [oq*2048, (oq+1)*2048) for tokens
    [th*4096, (th+1)*4096): x traffic halves vs pure output sharding; total
    per-core HBM traffic is 32(x) + 32(w both layouts) + 16(out bf16) = 80 MB.
  - Host feeds x transposed [d_in, tokens] and the w shard in BOTH layouts
    ([o, k] for the bit-exact per-row scale reduction on the DVE, [k, o] for
    quantization in matmul orientation) - layout choices only; all
    arithmetic happens on device.
  - Per-row scale uses the blocked-512 two-stage reduction (bit-exact match
    with the reference's jnp.mean lowering); w_q = (w > s/2) - (w < -s/2)
    equals clip(round(w/scale), -1, 1) exactly. Ternary w_q is exact in
    fp8e4 and bf16.
  - The full contraction (2048 = 16 chunks of 128) runs as fp8e4 DoubleRow
    matmuls (2 chunks/matmul, 2x PE rate); x is cast f32->fp8e4 on the
    scalar engine per 512-token slab. Measured end-to-end error 1.925e-2
    scale-relative (gate 2e-2), reproduced exactly by a host-side replica
    (deterministic inputs). BL_FP8_KC<16 falls back to bf16 tail chunks
    for extra margin.
  - Matmuls run with w_q stationary and x moving, so PSUM is [o, tokens]
    and the per-o scale is per-PARTITION: the epilogue is a scalar-engine
    activation copy (fast PSUM reads), keeping the vector engine free for
    quantization. Output is stored bf16 in [o, t] layout (contiguous rows,
    one 2 MB store per slab); the host transposes/upcasts.
  - DMA is spread over three queues (x loads on the sync HWDGE ring, weight
    loads on the gpsimd SWDGE ring, stores on the scalar HWDGE ring) so
    loads never queue behind stores.
  - Weight prologue is a ladder over o-quarters: quarter 0 quantizes first,
    then the first BL_EARLY slabs' quarter-q matmuls run on the PE while
    later quarters' scale/quantization fills the DVE gaps. Each quarter's
    scale prologue runs one phase ahead of its quant pass and quant work is
    front-loaded, so the strict-FIFO PE stream never convoys on the DVE
    quant producer.
"""

import os

import numpy as np

B, S, D_IN, D_OUT = 4, 2048, 2048, 8192
T = B * S  # 8192 tokens
N_CORES = 8
TH, OQN = 2, 4  # grid: token halves x out quarters
T_C = T // TH  # 4096 tokens per core
O_SHARD = D_OUT // OQN  # 2048 out features per core
EPS = 1e-05

P = 128
KC = D_IN // P  # 16 contraction chunks
T_SLAB = 512
N_SLABS = T_C // T_SLAB  # 8
N_OT = O_SHARD // P  # 16 o-tiles
OQ = 512  # o-columns per quant quarter
NQ = O_SHARD // OQ  # 4

NF8 = int(os.environ.get("BL_FP8_KC", "16"))  # k-chunks through fp8 DoubleRow
assert 0 <= NF8 <= KC and NF8 % 2 == 0
NB16 = KC - NF8
NPAIR = NF8 // 2
EARLY = int(os.environ.get("BL_EARLY", "6"))
EARLY = max(1, min(EARLY, N_SLABS))

_CACHE = {}


def _build_program():
    import concourse.bass as bass
    import concourse.tile as tile
    from concourse import bacc, mybir
    from concourse.masks import make_identity

    f32 = mybir.dt.float32
    bf16 = mybir.dt.bfloat16
    fp8 = mybir.dt.float8e4
    DR = mybir.MatmulPerfMode.DoubleRow

    nc = bacc.Bacc(
        "TRN2",
        target_bir_lowering=False,
        debug=False,
        num_devices=N_CORES,
    )

    xT = nc.dram_tensor("xT", [D_IN, T_C], f32, kind="ExternalInput")
    w = nc.dram_tensor("w", [O_SHARD, D_IN], f32, kind="ExternalInput")
    wT = nc.dram_tensor("wT", [D_IN, O_SHARD], f32, kind="ExternalInput")
    out = nc.dram_tensor("out", [O_SHARD, T_C], bf16, kind="ExternalOutput")

    xT3 = xT.ap().rearrange("(c p) t -> p c t", p=P)  # [128, 16, 4096]
    wT3 = wT.ap().rearrange("(c p) o -> p c o", p=P)  # [128, 16, 2048]
    out3 = out.ap().rearrange("(ot p) t -> p ot t", p=P)  # [128, 16, 4096]

    with tile.TileContext(nc) as tc:
        const_pool = tc.alloc_tile_pool(name="const", bufs=1)
        wq8_pool = tc.alloc_tile_pool(name="wq8", bufs=1)
        wq16_pool = tc.alloc_tile_pool(name="wq16", bufs=1)
        sb_pool = tc.alloc_tile_pool(name="thr", bufs=1)
        w_pool = tc.alloc_tile_pool(name="wstage", bufs=2)
        wt_pool = tc.alloc_tile_pool(name="wtstage", bufs=3)
        gl_pool = tc.alloc_tile_pool(name="gl", bufs=2)
        st_pool = tc.alloc_tile_pool(name="stats", bufs=4)
        psum_pro = tc.alloc_tile_pool(name="psum_pro", bufs=1, space="PSUM")
        xb_pool = tc.alloc_tile_pool(name="xb", bufs=EARLY + 1)
        xf_pool = tc.alloc_tile_pool(name="xf", bufs=3)
        out_pool = tc.alloc_tile_pool(name="osb", bufs=2)
        outh_pool = tc.alloc_tile_pool(name="osbh", bufs=2)
        psum_mm = tc.alloc_tile_pool(name="psum_mm", bufs=3, space="PSUM")
        ctx_pools = [const_pool, wq8_pool, wq16_pool, sb_pool, w_pool, wt_pool,
                     gl_pool, st_pool, psum_pro, xb_pool, xf_pool, out_pool,
                     outh_pool, psum_mm]

        ident_f32 = const_pool.tile([P, P], f32)
        make_identity(nc, ident_f32)
        ones_f32 = const_pool.tile([P, P], f32)
        nc.vector.memset(ones_f32[:], 1.0)

        # resident: ternary weights in [k, o] matmul orientation (fp8 chunks
        # + bf16 tail chunks), per-o-tile scale columns, and the +-threshold
        # broadcast across partitions (for quantization compares)
        wqT8 = wq8_pool.tile([P, NF8, O_SHARD], fp8, name="wqT8")
        wqT16 = (wq16_pool.tile([P, NB16, O_SHARD], bf16, name="wqT16")
                 if NB16 else None)
        scales = sb_pool.tile([P, N_OT], f32, tag="scales", name="scales")
        thrB = sb_pool.tile([P, O_SHARD], f32, tag="thrB", name="thrB")
        nthrB = sb_pool.tile([P, O_SHARD], f32, tag="nthrB", name="nthrB")

        def prologue_otile(ot):
            """Bit-exact per-row scale for o-tile `ot` + threshold broadcast."""
            wf = w_pool.tile([P, D_IN], f32, name="wf")
            eng = nc.gpsimd if ot < 4 else nc.scalar
            eng.dma_start(wf[:], w[bass.ts(ot, P), :])

            # blocked-512 two-stage reduce: bit-exact match with the
            # reference's lowered jnp.mean
            ssum4 = st_pool.tile([P, 4], f32, tag="ssum4", name="ssum4")
            nc.vector.tensor_reduce(
                out=ssum4[:],
                in_=wf[:].rearrange("p (b k) -> p b k", k=512),
                op=mybir.AluOpType.add,
                axis=mybir.AxisListType.X,
                apply_absolute_value=True,
            )
            ssum = st_pool.tile([P, 1], f32, tag="ssum", name="ssum")
            nc.vector.tensor_reduce(
                out=ssum[:], in_=ssum4[:],
                op=mybir.AluOpType.add, axis=mybir.AxisListType.X,
            )
            nc.vector.tensor_scalar(
                scales[:, bass.ds(ot, 1)], ssum[:], 1.0 / D_IN, EPS,
                mybir.AluOpType.mult, mybir.AluOpType.max,
            )
            thr = st_pool.tile([P, 1], f32, tag="thr", name="thr")
            nc.vector.tensor_scalar_mul(thr[:], scales[:, bass.ds(ot, 1)], 0.5)

            # thrB[:, ot*128:+128] = thr broadcast over partitions
            # (ones.T @ diag(thr)), likewise -thr into nthrB
            for sign, dst in ((1.0, thrB), (-1.0, nthrB)):
                ds_t = st_pool.tile([P, P], f32, tag=f"diag{sign}", name="ds_t")
                nc.vector.tensor_scalar(
                    ds_t[:], ident_f32[:], thr[:], sign,
                    mybir.AluOpType.mult, mybir.AluOpType.mult,
                )
                bp = psum_pro.tile([P, P], f32, tag="bp", name="bp", bufs=2)
                nc.tensor.matmul(bp[:], ones_f32[:], ds_t[:],
                                 start=True, stop=True)
                nc.scalar.copy(out=dst[:, bass.ts(ot, P)], in_=bp[:])

        def quant_tile(q, kc):
            """Quantize wT chunk kc, o-quarter q, into wqT8/wqT16."""
            qc = bass.ds(q * OQ, OQ)
            wtf = wt_pool.tile([P, OQ], f32, name="wtf")
            nc.gpsimd.dma_start(wtf[:], wT3[:, kc, qc])
            g = gl_pool.tile([P, OQ], f32, tag="g", name="g")
            nc.vector.tensor_tensor(g[:], wtf[:], thrB[:, qc],
                                    mybir.AluOpType.is_gt)
            l = gl_pool.tile([P, OQ], f32, tag="l", name="l")
            nc.vector.tensor_tensor(l[:], wtf[:], nthrB[:, qc],
                                    mybir.AluOpType.is_lt)
            dst = wqT8[:, kc, qc] if kc < NF8 else wqT16[:, kc - NF8, qc]
            nc.vector.tensor_tensor(dst, g[:], l[:], mybir.AluOpType.subtract)
        xb_tiles = {}

        def load_slab(s):
            tsl = bass.ts(s, T_SLAB)
            xb8 = xb_pool.tile([P, NF8, T_SLAB], fp8, tag="xb8", name="xb8")
            xb16 = (xb_pool.tile([P, NB16, T_SLAB], bf16, tag="xb16",
                                 name="xb16") if NB16 else None)
            for quar in range(4):
                xf = xf_pool.tile([P, 4, T_SLAB], f32, name="xf")
                nc.sync.dma_start(xf[:], xT3[:, bass.ts(quar, 4), tsl])
                base = quar * 4
                n8 = max(0, min(4, NF8 - base))
                if n8:
                    nc.scalar.copy(out=xb8[:, bass.ds(base, n8), :],
                                   in_=xf[:, bass.ds(0, n8), :])
                if n8 < 4:
                    nc.scalar.copy(
                        out=xb16[:, bass.ds(base + n8 - NF8, 4 - n8), :],
                        in_=xf[:, bass.ds(n8, 4 - n8), :])
            xb_tiles[s] = (xb8, xb16)

        def mm_group(ot, s, ps, side):
            """All matmuls for o-tile `ot` x token slab `s` (512 tokens)."""
            xb8, xb16 = xb_tiles[s]
            otc = bass.ts(ot, P)
            dst = ps[:, bass.ds(side * T_SLAB, T_SLAB)]
            n_mm = NPAIR + NB16
            idx = 0
            for kp in range(NPAIR):
                nc.tensor.matmul(
                    dst,
                    wqT8[:, bass.ds(2 * kp, 2), otc],
                    xb8[:, bass.ds(2 * kp, 2), :],
                    start=(idx == 0),
                    stop=(idx == n_mm - 1),
                    perf_mode=DR,
                )
                idx += 1
            for j in range(NB16):
                nc.tensor.matmul(
                    dst,
                    wqT16[:, j, otc],
                    xb16[:, j, :],
                    start=(idx == 0),
                    stop=(idx == n_mm - 1),
                )
                idx += 1

        def ot_pair(ot, s, osb, slot):
            """Two o-tiles x one slab through one 2-bank psum tile."""
            ps = psum_mm.tile([P, 2 * T_SLAB], f32, tag="ps", name="ps")
            mm_group(ot, s, ps, 0)
            mm_group(ot + 1, s, ps, 1)
            for i in (0, 1):
                nc.scalar.mul(osb[:, slot + i, :],
                              ps[:, bass.ds(i * T_SLAB, T_SLAB)],
                              scales[:, bass.ds(ot + i, 1)])

        # ---------------- emission schedule -----------------------------
        load_slab(0)
        for ot in range(4):
            prologue_otile(ot)
        for kc in range(KC):
            quant_tile(0, kc)
        for s in range(1, EARLY):
            load_slab(s)

        # Ladder over o-quarters: run the early slabs' quarter-q groups
        # while quarter q+1's prologue fills the DVE gaps.
        # each quarter's scale prologue runs one phase ahead of its quant
        # pass, so quant q+1 can start the moment phase q begins
        phase_tasks = {
            0: ([lambda ot=ot: prologue_otile(ot) for ot in range(4, 12)]
                + [lambda kc=kc: quant_tile(1, kc) for kc in range(KC)]),
            1: ([lambda ot=ot: prologue_otile(ot) for ot in range(12, 16)]
                + [lambda kc=kc: quant_tile(2, kc) for kc in range(KC)]
                + [lambda kc=kc: quant_tile(3, kc) for kc in range(KC // 2)]),
            2: [lambda kc=kc: quant_tile(3, kc) for kc in range(KC // 2, KC)],
            3: [],
        }
        for q in range(NQ):
            tasks = phase_tasks[q]
            units = [(s, 4 * q + 2 * pi) for s in range(EARLY)
                     for pi in range(2)]
            per = (3 * len(tasks) + 2 * len(units) - 1) // (2 * len(units))
            ci = 0
            osbs = {}
            for (s, ot) in units:
                for _ in range(per):
                    if ci < len(tasks):
                        tasks[ci]()
                        ci += 1
                if s not in osbs:
                    osbs[s] = outh_pool.tile([P, 4, T_SLAB], bf16,
                                             name="osbh")
                ot_pair(ot, s, osbs[s], ot - 4 * q)
                if ot % 4 == 2:  # second pair of the quarter for this slab
                    nc.scalar.dma_start(
                        out3[:, bass.ds(4 * q, 4), bass.ts(s, T_SLAB)],
                        osbs.pop(s)[:])
            while ci < len(tasks):
                tasks[ci]()
                ci += 1

        # steady state: all 16 o-tiles per slab, one batched 2 MB store
        for s in range(EARLY, N_SLABS):
            load_slab(s)
            osb = out_pool.tile([P, N_OT, T_SLAB], bf16, name="osb")
            for pi in range(N_OT // 2):
                ot_pair(2 * pi, s, osb, 2 * pi)
            nc.scalar.dma_start(out3[:, :, bass.ts(s, T_SLAB)], osb[:])

        for p in reversed(ctx_pools):
            p.release()

    nc.compile()
    return nc


def _get_program():
    if "nc" not in _CACHE:
        _CACHE["nc"] = _build_program()
    return _CACHE["nc"]


def _ensure_ntff_hook():
    """Provide antenv.axon_hooks if the image lacks it (profiling only)."""
    import sys
    import types

    try:
        from antenv.axon_hooks import get_axon_ntff_profile_hook  # noqa: F401
        return
    except ImportError:
        pass
    try:
        import antenv
        from trn_agent_boot.trn_boot import _ntff_profile_via_ctypes

        mod = types.ModuleType("antenv.axon_hooks")
        state = {"hook": _ntff_profile_via_ctypes("/opt/axon/libaxon_pjrt.so")}
        mod.get_axon_ntff_profile_hook = lambda: state["hook"]
        mod.set_axon_ntff_profile_hook = lambda h: state.__setitem__("hook", h)
        sys.modules["antenv.axon_hooks"] = mod
        antenv.axon_hooks = mod
    except Exception:
        pass


def kernel(x: np.ndarray, weight: np.ndarray) -> np.ndarray:
    from concourse.bass_utils import run_bass_kernel_spmd

    assert x.shape == (B, S, D_IN) and weight.shape == (D_OUT, D_IN)
    nc = _get_program()

    xT = np.ascontiguousarray(x.reshape(T, D_IN).T)  # [D_IN, T]
    in_maps = []
    for c in range(N_CORES):
        th, oq = divmod(c, OQN)
        w_shard = weight[oq * O_SHARD:(oq + 1) * O_SHARD]
        in_maps.append({
            "xT": np.ascontiguousarray(xT[:, th * T_C:(th + 1) * T_C]),
            "w": w_shard,
            "wT": np.ascontiguousarray(w_shard.T),
        })

    trace = os.environ.get("BL_TRACE", "0") == "1"
    if trace:
        _ensure_ntff_hook()
    res = run_bass_kernel_spmd(nc, in_maps, list(range(N_CORES)), trace=trace)
    _CACHE["last_results"] = res

    fullT = np.empty((D_OUT, T), dtype=np.float32)  # [o, t]
    for c in range(N_CORES):
        th, oq = divmod(c, OQN)
        part = np.asarray(res.results[c]["out"]).astype(np.float32)
        fullT[oq * O_SHARD:(oq + 1) * O_SHARD, th * T_C:(th + 1) * T_C] = part
    return np.ascontiguousarray(fullT.T.reshape(B, S, D_OUT))
